# revision 11
# baseline (speedup 1.0000x reference)
"""Distributed FWHT (Hamiltonian -> Pauli-string coefficients) on 8 TRN2 cores, v13.

Wall-clock-focused design. The axon tunnel moves ~30-37MB/s up and
~30-65MB/s down, so wire bytes dominate wall time; device compute is
~100x smaller. Measured-driven choices:

  - uint8 wire format both directions with per-(row, 2048-col-block)
    scales (absmax/126): ~16.4MB up + ~16.4MB down total. Input is
    dequantized on-device ((q-128)*s per block); output re-quantized
    on-device with an exact-integer trick (bias 2^23+128, then subtract
    2^23) so the f32->u8 conversion is exact regardless of rounding mode
    (probe measured round-to-nearest; this path is mode-independent).
  - scales ride INSIDE the u8 tensors as 32 extra bitcast columns --
    sharded device_put/fetch calls have ~50-100ms fixed cost each, so
    one put up and one array down is the minimum.
  - donated output buffers are recycled device arrays (previous call's
    output; jitted device-side zeros on the first call): the naive path
    uploads 16MB of host zeros per call.
  - the jitted shard_map executable is built ONCE and cached
    (run_bass_kernel_spmd re-traces and re-jits on every call).
  - download is streamed per-shard (copy_to_host_async + per-shard
    asarray) with dequantization interleaved, hiding host dequant.
  - local FWHT runs as three rotating matmul passes contracting
    a -> b -> c (local index bits a7|b7|c7) with strided stationary
    operands, so after the cross-core combine and a SECOND AllToAll each
    core holds its contiguous 2^21-element output slab in row-major
    [a'|b'|c'] order: host gather is concat + dequant, no transpose.

Layout per core j (x_j = x[j*2^21:(j+1)*2^21] as [128, 16384]):
  pass1 (contract a): lhsT = X[:, c0::128]  -> T1[b, (c, a')]
  pass2 (contract b): lhsT = T1[:, a0::128] -> T2[c, (a', b')]
  pass3 (contract c): lhsT = T2[:, b0::128] -> F3[a', (b', c')]
  A2A#1 over a' high-3 bits, combine with kron(H8,I16)/8 over source
  cores, A2A#2 over j' high-3 bits -> G[a', (b', c')] = slab j'=core.
Scaling 1/2^24 folded into Hs=H/128 (x3) and M=kron(H8,I16)/8.
"""

import numpy as np
import ml_dtypes

NCORES = 8
P = 128
F = 16384
NBLK = 8            # scale blocks per row
BLK = F // NBLK     # 2048 cols per scale block
SCOLS = NBLK * 4    # 32 u8 columns carrying NBLK f32 scales
FT = F + SCOLS      # wire tensor width
LOCAL = P * F

_BIG = 8388608.0    # 2^23: f32 add forces round-to-integer


def _hadamard(n: int) -> np.ndarray:
    H = np.array([[1.0]], dtype=np.float64)
    while H.shape[0] < n:
        H = np.block([[H, H], [H, -H]])
    return H


_CACHE: dict = {}


def _build_module():
    import concourse.mybir as mybir
    import concourse.tile as tile
    from concourse import bacc

    f32 = mybir.dt.float32
    bf16 = mybir.dt.bfloat16
    u8 = mybir.dt.uint8
    Copy = mybir.ActivationFunctionType.Copy

    Hs_np = (_hadamard(128) / 128.0).astype(ml_dtypes.bfloat16)
    M_np = (np.kron(_hadamard(8), np.eye(16)) / 8.0).astype(ml_dtypes.bfloat16)

    nc = bacc.Bacc(
        "TRN2",
        target_bir_lowering=False,
        debug=False,
        enable_asserts=False,
        num_devices=NCORES,
    )

    x_in = nc.dram_tensor("x", [P, FT], u8, kind="ExternalInput")
    y_out = nc.dram_tensor("y", [P, FT], u8, kind="ExternalOutput")
    Hs_dram = nc.inline_tensor(Hs_np, name="Hs_const")
    M_dram = nc.inline_tensor(M_np, name="M_const")

    with tile.TileContext(nc) as tc:
        with (
            tc.tile_pool(name="big", bufs=1) as big,
            tc.tile_pool(name="small", bufs=1) as small,
            tc.tile_pool(name="tmp", bufs=2) as tmp,
            tc.tile_pool(name="psum", bufs=8, space="PSUM") as psum,
            tc.tile_pool(name="dram", bufs=1, space="DRAM") as dram,
        ):
            Hs_t = small.tile([P, 128], bf16, tag="hs")
            M_t = small.tile([P, 128], bf16, tag="m")
            s_u8 = small.tile([P, SCOLS], u8, tag="s")
            nc.sync.dma_start(Hs_t[:], Hs_dram[:])
            nc.sync.dma_start(M_t[:], M_dram[:])
            nc.sync.dma_start(s_u8[:], x_in[:, F:FT])
            s_f32 = s_u8[:].bitcast(f32)      # [P, NBLK] per-row scales

            Xq = big.tile([P, F], u8, tag="xq")
            for h in range(2):
                eng = nc.sync if h == 0 else nc.scalar
                eng.dma_start(
                    Xq[:, h * (F // 2): (h + 1) * (F // 2)],
                    x_in[:, h * (F // 2): (h + 1) * (F // 2)],
                )

            # dequant u8 -> bf16: (q - 128) * s_blk, per 2048-col block
            A = big.tile([P, F], bf16, tag="A")      # Xb -> T2 -> V -> G
            for blk in range(NBLK):
                sl = slice(blk * BLK, (blk + 1) * BLK)
                nc.vector.tensor_scalar(
                    A[:, sl], Xq[:, sl], -128.0, s_f32[:, blk: blk + 1],
                    op0=mybir.AluOpType.add, op1=mybir.AluOpType.mult,
                )

            B = big.tile([P, F], bf16, tag="B")      # T1 -> F3 -> O

            def fwht_pass(src_ap, dst_tile):
                # contract the partition axis: 128 strided stationary blocks
                for g in range(32):
                    pt = psum.tile([P, 512], f32, tag="ps")
                    for jj in range(4):
                        k = g * 4 + jj
                        nc.tensor.matmul(
                            pt[:, jj * 128: (jj + 1) * 128],
                            src_ap[:, k, :],
                            Hs_t[:],
                        )
                    eng = nc.vector.tensor_copy if g % 2 == 0 else nc.scalar.copy
                    eng(dst_tile[:, g * 512: (g + 1) * 512], pt[:])

            # pass1: contract a; lhsT = Xb[:, c0::128] ([a,b]) -> T1[b,(c,a')]
            fwht_pass(A[:].rearrange("p (b c) -> p c b", c=128), B)
            # pass2: contract b; lhsT = T1[:, a0::128] ([b,c]) -> T2[c,(a',b')]
            T2 = big.tile([P, F], bf16, tag="A")
            fwht_pass(B[:].rearrange("p (c a) -> p a c", a=128), T2)
            # pass3: contract c; lhsT = T2[:, b0::128] ([c,a']) -> F3[a',(b',c')]
            F3 = big.tile([P, F], bf16, tag="B")
            fwht_pass(T2[:].rearrange("p (a b) -> p b a", b=128), F3)

            # A2A#1 over a' high-3 bits
            a2a1_in = dram.tile([P, F], bf16, tag="a2a1i", name="a2a1_in")
            a2a1_out = dram.tile([P, F], bf16, tag="a2a1o", name="a2a1_out")
            nc.sync.dma_start(a2a1_in[:], F3[:])
            nc.gpsimd.collective_compute(
                "AllToAll",
                mybir.AluOpType.bypass,
                replica_groups=[list(range(NCORES))],
                ins=[a2a1_in.opt()],
                outs=[a2a1_out.opt()],
            )
            V = big.tile([P, F], bf16, tag="A")
            nc.scalar.dma_start(V[:], a2a1_out[:])

            # combine over source cores: O = M^T V
            O = big.tile([P, F], bf16, tag="B")
            for g in range(32):
                pt = psum.tile([P, 512], f32, tag="ps")
                nc.tensor.matmul(pt[:], M_t[:], V[:, g * 512: (g + 1) * 512])
                eng = nc.vector.tensor_copy if g % 2 == 0 else nc.scalar.copy
                eng(O[:, g * 512: (g + 1) * 512], pt[:])

            # A2A#2 over j' high-3 bits -> full contiguous slab
            a2a2_in = dram.tile([P, F], bf16, tag="a2a2i", name="a2a2_in")
            a2a2_out = dram.tile([P, F], bf16, tag="a2a2o", name="a2a2_out")
            nc.sync.dma_start(a2a2_in[:], O[:])
            nc.gpsimd.collective_compute(
                "AllToAll",
                mybir.AluOpType.bypass,
                replica_groups=[list(range(NCORES))],
                ins=[a2a2_in.opt()],
                outs=[a2a2_out.opt()],
            )
            G = big.tile([P, F], bf16, tag="A")
            nc.scalar.dma_start(G[:], a2a2_out[:])

            # per-(row, block) re-quantization to u8
            Qu = big.tile([P, F], u8, tag="qu")
            so_t = small.tile([P, NBLK], f32, tag="so")
            for blk in range(NBLK):
                sl = slice(blk * BLK, (blk + 1) * BLK)
                am = small.tile([P, 1], f32, tag=f"am{blk}")
                rc = small.tile([P, 1], f32, tag=f"rc{blk}")
                rs = small.tile([P, 1], f32, tag=f"rs{blk}")
                nc.vector.tensor_reduce(
                    out=am[:], in_=G[:, sl], op=mybir.AluOpType.max,
                    axis=mybir.AxisListType.X, apply_absolute_value=True,
                )
                nc.vector.reciprocal(rc[:], am[:])
                nc.vector.tensor_scalar(
                    rs[:], rc[:], 126.0, None, op0=mybir.AluOpType.mult,
                )
                nc.vector.tensor_scalar(
                    so_t[:, blk: blk + 1], am[:], 1.0 / 126.0, None,
                    op0=mybir.AluOpType.mult,
                )
                u = tmp.tile([P, BLK], f32, tag="u")
                nc.scalar.activation(
                    u[:], G[:, sl], Copy, bias=_BIG + 128.0, scale=rs[:],
                )
                nc.vector.tensor_scalar(
                    Qu[:, sl], u[:], -_BIG, None,
                    op0=mybir.AluOpType.add,
                )
            nc.scalar.dma_start(y_out[:, F:FT], so_t[:].bitcast(u8))
            for h in range(2):
                eng = nc.scalar if h == 0 else nc.sync
                eng.dma_start(
                    y_out[:, h * (F // 2): (h + 1) * (F // 2)],
                    Qu[:, h * (F // 2): (h + 1) * (F // 2)],
                )

    nc.compile()
    return nc


class _Res:
    exec_time_ns = None


def _get_exec():
    if "exec" in _CACHE:
        return _CACHE["exec"]

    import jax
    import jax.numpy as jnp
    from jax.sharding import Mesh, PartitionSpec, NamedSharding
    from jax.experimental.shard_map import shard_map
    import concourse.mybir as mybir
    from concourse import bass2jax

    nc = _build_module()
    bass2jax.install_neuronx_cc_hook()

    partition_name = (
        nc.partition_id_tensor.name if nc.partition_id_tensor else None
    )

    in_names: list[str] = []
    out_names: list[str] = []
    out_avals = []
    out_shapes = []
    for alloc in nc.m.functions[0].allocations:
        if not isinstance(alloc, mybir.MemoryLocationSet):
            continue
        name = alloc.memorylocations[0].name
        if alloc.kind == "ExternalInput":
            if name != partition_name:
                in_names.append(name)
        elif alloc.kind == "ExternalOutput":
            out_names.append(name)
            shape = tuple(alloc.tensor_shape)
            dtype = mybir.dt.np(alloc.dtype)
            out_avals.append(jax.core.ShapedArray(shape, dtype))
            out_shapes.append((shape, dtype))
    n_params = len(in_names)
    n_outs = len(out_names)
    all_in_names = list(in_names) + list(out_names)
    if partition_name is not None:
        all_in_names.append(partition_name)

    from concourse.bass2jax import _bass_exec_p, partition_id_tensor

    out_avals_t = tuple(out_avals)
    all_in_names_t = tuple(all_in_names)
    out_names_t = tuple(out_names)

    def _body(*args):
        operands = list(args)
        if partition_name is not None:
            operands.append(partition_id_tensor())
        outs = _bass_exec_p.bind(
            *operands,
            out_avals=out_avals_t,
            in_names=all_in_names_t,
            out_names=out_names_t,
            lowering_input_output_aliases=(),
            sim_require_finite=True,
            sim_require_nnan=True,
            nc=nc,
        )
        return tuple(outs)

    devices = jax.devices()[:NCORES]
    mesh = Mesh(np.asarray(devices), ("core",))
    spec = PartitionSpec("core")
    sh = NamedSharding(mesh, spec)
    in_specs = (spec,) * (n_params + n_outs)
    out_specs = (spec,) * n_outs
    donate = tuple(range(n_params, n_params + n_outs))
    sharded = jax.jit(
        shard_map(
            _body, mesh=mesh, in_specs=in_specs, out_specs=out_specs,
            check_rep=False,
        ),
        donate_argnums=donate,
        keep_unused=True,
    )

    def zeros_fn_py():
        return tuple(
            jnp.zeros((NCORES * shp[0], *shp[1:]), dt)
            for (shp, dt) in out_shapes
        )

    zeros_fn = jax.jit(zeros_fn_py, out_shardings=(sh,) * n_outs)

    state = {
        "nc": nc,
        "sharded": sharded,
        "zeros_fn": zeros_fn,
        "sh": sh,
        "prev": None,
        "xbuf": np.empty((NCORES * P, FT), np.uint8),
        "qf": np.empty((NCORES * P, NBLK, BLK), np.float32),
        "fbuf": np.empty((P, NBLK, BLK), np.float32),
        "ybuf": np.empty(NCORES * LOCAL, np.float32),
    }
    _CACHE["exec"] = state
    return state


def _quantize(x: np.ndarray, xbuf: np.ndarray, qf: np.ndarray) -> None:
    """Quantize x into the preallocated wire buffer xbuf [1024, FT] u8.

    numpy ties a fused numba kernel here (~37ms; memory-bandwidth-bound on
    this single-CPU host), so stay dependency-free.
    """
    xr = x.reshape(NCORES * P, NBLK, BLK)
    # absmax via max/-min: two reduction passes, no 64MB |x| temp
    absmax = np.maximum(xr.max(axis=2), -xr.min(axis=2))
    np.maximum(absmax, np.float32(1e-30), out=absmax)
    s = (absmax / np.float32(126.0)).astype(np.float32)
    inv = np.float32(1.0) / s
    np.multiply(xr, inv[..., None], out=qf)
    qf += np.float32(128.5)
    np.copyto(
        xbuf[:, :F].reshape(NCORES * P, NBLK, BLK), qf, casting="unsafe"
    )  # trunc of positive = floor = round-half-up
    xbuf[:, F:FT] = s.view(np.uint8).reshape(NCORES * P, SCOLS)


def _dequant_shard(Yj: np.ndarray, out: np.ndarray, fbuf: np.ndarray) -> None:
    """Dequantize one core's wire shard Yj [P, FT] u8 into out [LOCAL] f32."""
    sj = Yj[:, F:FT].copy().view(np.float32).reshape(P, NBLK, 1)
    np.copyto(fbuf, Yj[:, :F].reshape(P, NBLK, BLK), casting="unsafe")
    fbuf -= np.float32(128.0)
    fbuf *= sj
    out[:] = fbuf.reshape(-1)


def run(x: np.ndarray, trace: bool = False):
    import jax

    st = _get_exec()
    x = np.ascontiguousarray(x, dtype=np.float32)
    assert x.shape == (NCORES * LOCAL,)

    _quantize(x, st["xbuf"], st["qf"])
    Xd = jax.device_put(st["xbuf"], st["sh"])

    bufs = st["zeros_fn"]() if st["prev"] is None else st["prev"]
    outs = st["sharded"](Xd, *bufs)
    st["prev"] = outs

    y_g = outs[0]
    y_g.copy_to_host_async()
    yv = st["ybuf"]
    shards = sorted(y_g.addressable_shards, key=lambda sh_: sh_.index[0].start)
    for j, shd in enumerate(shards):
        Yj = np.asarray(shd.data)
        _dequant_shard(Yj, yv[j * LOCAL: (j + 1) * LOCAL], st["fbuf"])
    # yv is a reused buffer: safe because callers consume the result before
    # invoking run() again (and kernel() returns the final attempt's view).
    return yv, _Res()


def kernel(Hamiltonian: np.ndarray) -> np.ndarray:
    # Validate cheaply on the host (no NaN + Parseval for the orthogonal
    # scaled transform) and retry on rare infra flakes.
    x = np.ascontiguousarray(Hamiltonian, dtype=np.float32)
    ref_norm2 = float(np.square(x, dtype=np.float64).sum()) / (NCORES * LOCAL)
    y = None
    for _attempt in range(3):
        try:
            y, _ = run(x, trace=False)
        except Exception:
            _CACHE.pop("exec", None)
            if _attempt == 2 and y is None:
                raise
            continue
        if np.isnan(y).any():
            continue
        norm2 = float(np.square(y, dtype=np.float64).sum())
        if abs(norm2 - ref_norm2) <= 0.02 * ref_norm2:
            break
    # run() returns a reused buffer view; hand the caller an owned copy.
    return None if y is None else y.copy()


# revision 14
# speedup vs baseline: 1.0779x; 1.0779x over previous
"""Distributed FWHT (Hamiltonian -> Pauli-string coefficients) on 8 TRN2 cores, v13.

Wall-clock-focused design. The axon tunnel moves ~30-37MB/s up and
~30-65MB/s down, so wire bytes dominate wall time; device compute is
~100x smaller. Measured-driven choices:

  - uint8 wire format both directions with per-(row, 2048-col-block)
    scales (absmax/126): ~16.4MB up + ~16.4MB down total. Input is
    dequantized on-device ((q-128)*s per block); output re-quantized
    on-device with an exact-integer trick (bias 2^23+128, then subtract
    2^23) so the f32->u8 conversion is exact regardless of rounding mode
    (probe measured round-to-nearest; this path is mode-independent).
  - scales ride INSIDE the u8 tensors as 32 extra bitcast columns --
    sharded device_put/fetch calls have ~50-100ms fixed cost each, so
    one put up and one array down is the minimum.
  - donated output buffers are recycled device arrays (previous call's
    output; jitted device-side zeros on the first call): the naive path
    uploads 16MB of host zeros per call.
  - the jitted shard_map executable is built ONCE and cached
    (run_bass_kernel_spmd re-traces and re-jits on every call).
  - download is streamed per-shard (copy_to_host_async + per-shard
    asarray) with dequantization interleaved, hiding host dequant.
  - local FWHT runs as three rotating matmul passes contracting
    a -> b -> c (local index bits a7|b7|c7) with strided stationary
    operands, so after the cross-core combine and a SECOND AllToAll each
    core holds its contiguous 2^21-element output slab in row-major
    [a'|b'|c'] order: host gather is concat + dequant, no transpose.

Layout per core j (x_j = x[j*2^21:(j+1)*2^21] as [128, 16384]):
  pass1 (contract a): lhsT = X[:, c0::128]  -> T1[b, (c, a')]
  pass2 (contract b): lhsT = T1[:, a0::128] -> T2[c, (a', b')]
  pass3 (contract c): lhsT = T2[:, b0::128] -> F3[a', (b', c')]
  A2A#1 over a' high-3 bits, combine with kron(H8,I16)/8 over source
  cores, A2A#2 over j' high-3 bits -> G[a', (b', c')] = slab j'=core.
Scaling 1/2^24 folded into Hs=H/128 (x3) and M=kron(H8,I16)/8.
"""

import numpy as np
import ml_dtypes

NCORES = 8
P = 128
F = 16384
NBLK = 8            # scale blocks per row
BLK = F // NBLK     # 2048 cols per scale block
SCOLS = NBLK * 4    # 32 u8 columns carrying NBLK f32 scales
FT = F + SCOLS      # wire tensor width
LOCAL = P * F

_BIG = 8388608.0    # 2^23: f32 add forces round-to-integer


def _hadamard(n: int) -> np.ndarray:
    H = np.array([[1.0]], dtype=np.float64)
    while H.shape[0] < n:
        H = np.block([[H, H], [H, -H]])
    return H


_CACHE: dict = {}


def _build_module():
    import concourse.mybir as mybir
    import concourse.tile as tile
    from concourse import bacc

    f32 = mybir.dt.float32
    bf16 = mybir.dt.bfloat16
    u8 = mybir.dt.uint8
    Copy = mybir.ActivationFunctionType.Copy

    Hs_np = (_hadamard(128) / 128.0).astype(ml_dtypes.bfloat16)
    M_np = (np.kron(_hadamard(8), np.eye(16)) / 8.0).astype(ml_dtypes.bfloat16)

    nc = bacc.Bacc(
        "TRN2",
        target_bir_lowering=False,
        debug=False,
        enable_asserts=False,
        num_devices=NCORES,
    )

    x_in = nc.dram_tensor("x", [P, FT], u8, kind="ExternalInput")
    y_out = nc.dram_tensor("y", [P, FT], u8, kind="ExternalOutput")
    Hs_dram = nc.inline_tensor(Hs_np, name="Hs_const")
    M_dram = nc.inline_tensor(M_np, name="M_const")

    with tile.TileContext(nc) as tc:
        with (
            tc.tile_pool(name="big", bufs=1) as big,
            tc.tile_pool(name="small", bufs=1) as small,
            tc.tile_pool(name="tmp", bufs=2) as tmp,
            tc.tile_pool(name="psum", bufs=8, space="PSUM") as psum,
            tc.tile_pool(name="dram", bufs=1, space="DRAM") as dram,
        ):
            Hs_t = small.tile([P, 128], bf16, tag="hs")
            M_t = small.tile([P, 128], bf16, tag="m")
            s_u8 = small.tile([P, SCOLS], u8, tag="s")
            nc.sync.dma_start(Hs_t[:], Hs_dram[:])
            nc.sync.dma_start(M_t[:], M_dram[:])
            nc.sync.dma_start(s_u8[:], x_in[:, F:FT])
            s_f32 = s_u8[:].bitcast(f32)      # [P, NBLK] per-row scales

            Xq = big.tile([P, F], u8, tag="xq")
            for h in range(2):
                eng = nc.sync if h == 0 else nc.scalar
                eng.dma_start(
                    Xq[:, h * (F // 2): (h + 1) * (F // 2)],
                    x_in[:, h * (F // 2): (h + 1) * (F // 2)],
                )

            # dequant u8 -> bf16: (q - 128) * s_blk, per 2048-col block
            A = big.tile([P, F], bf16, tag="A")      # Xb -> T2 -> V -> G
            for blk in range(NBLK):
                sl = slice(blk * BLK, (blk + 1) * BLK)
                nc.vector.tensor_scalar(
                    A[:, sl], Xq[:, sl], -128.0, s_f32[:, blk: blk + 1],
                    op0=mybir.AluOpType.add, op1=mybir.AluOpType.mult,
                )

            B = big.tile([P, F], bf16, tag="B")      # T1 -> F3 -> O

            def fwht_pass(src_ap, dst_tile):
                # contract the partition axis: 128 strided stationary blocks
                for g in range(32):
                    pt = psum.tile([P, 512], f32, tag="ps")
                    for jj in range(4):
                        k = g * 4 + jj
                        nc.tensor.matmul(
                            pt[:, jj * 128: (jj + 1) * 128],
                            src_ap[:, k, :],
                            Hs_t[:],
                        )
                    eng = nc.vector.tensor_copy if g % 2 == 0 else nc.scalar.copy
                    eng(dst_tile[:, g * 512: (g + 1) * 512], pt[:])

            # pass1: contract a; lhsT = Xb[:, c0::128] ([a,b]) -> T1[b,(c,a')]
            fwht_pass(A[:].rearrange("p (b c) -> p c b", c=128), B)
            # pass2: contract b; lhsT = T1[:, a0::128] ([b,c]) -> T2[c,(a',b')]
            T2 = big.tile([P, F], bf16, tag="A")
            fwht_pass(B[:].rearrange("p (c a) -> p a c", a=128), T2)
            # pass3: contract c; lhsT = T2[:, b0::128] ([c,a']) -> F3[a',(b',c')]
            F3 = big.tile([P, F], bf16, tag="B")
            fwht_pass(T2[:].rearrange("p (a b) -> p b a", b=128), F3)

            # A2A#1 over a' high-3 bits
            a2a1_in = dram.tile([P, F], bf16, tag="a2a1i", name="a2a1_in")
            a2a1_out = dram.tile([P, F], bf16, tag="a2a1o", name="a2a1_out")
            nc.sync.dma_start(a2a1_in[:], F3[:])
            nc.gpsimd.collective_compute(
                "AllToAll",
                mybir.AluOpType.bypass,
                replica_groups=[list(range(NCORES))],
                ins=[a2a1_in.opt()],
                outs=[a2a1_out.opt()],
            )
            V = big.tile([P, F], bf16, tag="A")
            nc.scalar.dma_start(V[:], a2a1_out[:])

            # combine over source cores: O = M^T V
            O = big.tile([P, F], bf16, tag="B")
            for g in range(32):
                pt = psum.tile([P, 512], f32, tag="ps")
                nc.tensor.matmul(pt[:], M_t[:], V[:, g * 512: (g + 1) * 512])
                eng = nc.vector.tensor_copy if g % 2 == 0 else nc.scalar.copy
                eng(O[:, g * 512: (g + 1) * 512], pt[:])

            # A2A#2 over j' high-3 bits -> full contiguous slab
            a2a2_in = dram.tile([P, F], bf16, tag="a2a2i", name="a2a2_in")
            a2a2_out = dram.tile([P, F], bf16, tag="a2a2o", name="a2a2_out")
            nc.sync.dma_start(a2a2_in[:], O[:])
            nc.gpsimd.collective_compute(
                "AllToAll",
                mybir.AluOpType.bypass,
                replica_groups=[list(range(NCORES))],
                ins=[a2a2_in.opt()],
                outs=[a2a2_out.opt()],
            )
            G = big.tile([P, F], bf16, tag="A")
            nc.scalar.dma_start(G[:], a2a2_out[:])

            # per-(row, block) re-quantization to u8
            Qu = big.tile([P, F], u8, tag="qu")
            so_t = small.tile([P, NBLK], f32, tag="so")
            for blk in range(NBLK):
                sl = slice(blk * BLK, (blk + 1) * BLK)
                am = small.tile([P, 1], f32, tag=f"am{blk}")
                rc = small.tile([P, 1], f32, tag=f"rc{blk}")
                rs = small.tile([P, 1], f32, tag=f"rs{blk}")
                nc.vector.tensor_reduce(
                    out=am[:], in_=G[:, sl], op=mybir.AluOpType.max,
                    axis=mybir.AxisListType.X, apply_absolute_value=True,
                )
                nc.vector.reciprocal(rc[:], am[:])
                nc.vector.tensor_scalar(
                    rs[:], rc[:], 126.0, None, op0=mybir.AluOpType.mult,
                )
                nc.vector.tensor_scalar(
                    so_t[:, blk: blk + 1], am[:], 1.0 / 126.0, None,
                    op0=mybir.AluOpType.mult,
                )
                u = tmp.tile([P, BLK], f32, tag="u")
                nc.scalar.activation(
                    u[:], G[:, sl], Copy, bias=_BIG + 128.0, scale=rs[:],
                )
                nc.vector.tensor_scalar(
                    Qu[:, sl], u[:], -_BIG, None,
                    op0=mybir.AluOpType.add,
                )
            nc.scalar.dma_start(y_out[:, F:FT], so_t[:].bitcast(u8))
            for h in range(2):
                eng = nc.scalar if h == 0 else nc.sync
                eng.dma_start(
                    y_out[:, h * (F // 2): (h + 1) * (F // 2)],
                    Qu[:, h * (F // 2): (h + 1) * (F // 2)],
                )

    nc.compile()
    return nc


class _Res:
    exec_time_ns = None


def _get_exec():
    if "exec" in _CACHE:
        return _CACHE["exec"]

    import jax
    import jax.numpy as jnp
    from jax.sharding import Mesh, PartitionSpec, NamedSharding
    from jax.experimental.shard_map import shard_map
    import concourse.mybir as mybir
    from concourse import bass2jax

    nc = _build_module()
    bass2jax.install_neuronx_cc_hook()

    partition_name = (
        nc.partition_id_tensor.name if nc.partition_id_tensor else None
    )

    in_names: list[str] = []
    out_names: list[str] = []
    out_avals = []
    out_shapes = []
    for alloc in nc.m.functions[0].allocations:
        if not isinstance(alloc, mybir.MemoryLocationSet):
            continue
        name = alloc.memorylocations[0].name
        if alloc.kind == "ExternalInput":
            if name != partition_name:
                in_names.append(name)
        elif alloc.kind == "ExternalOutput":
            out_names.append(name)
            shape = tuple(alloc.tensor_shape)
            dtype = mybir.dt.np(alloc.dtype)
            out_avals.append(jax.core.ShapedArray(shape, dtype))
            out_shapes.append((shape, dtype))
    n_params = len(in_names)
    n_outs = len(out_names)
    all_in_names = list(in_names) + list(out_names)
    if partition_name is not None:
        all_in_names.append(partition_name)

    from concourse.bass2jax import _bass_exec_p, partition_id_tensor

    out_avals_t = tuple(out_avals)
    all_in_names_t = tuple(all_in_names)
    out_names_t = tuple(out_names)

    def _body(*args):
        operands = list(args)
        if partition_name is not None:
            operands.append(partition_id_tensor())
        outs = _bass_exec_p.bind(
            *operands,
            out_avals=out_avals_t,
            in_names=all_in_names_t,
            out_names=out_names_t,
            lowering_input_output_aliases=(),
            sim_require_finite=True,
            sim_require_nnan=True,
            nc=nc,
        )
        return tuple(outs)

    devices = jax.devices()[:NCORES]
    mesh = Mesh(np.asarray(devices), ("core",))
    spec = PartitionSpec("core")
    sh = NamedSharding(mesh, spec)
    in_specs = (spec,) * (n_params + n_outs)
    out_specs = (spec,) * n_outs
    donate = tuple(range(n_params, n_params + n_outs))
    sharded = jax.jit(
        shard_map(
            _body, mesh=mesh, in_specs=in_specs, out_specs=out_specs,
            check_rep=False,
        ),
        donate_argnums=donate,
        keep_unused=True,
    )

    def zeros_fn_py():
        return tuple(
            jnp.zeros((NCORES * shp[0], *shp[1:]), dt)
            for (shp, dt) in out_shapes
        )

    zeros_fn = jax.jit(zeros_fn_py, out_shardings=(sh,) * n_outs)

    state = {
        "nc": nc,
        "sharded": sharded,
        "zeros_fn": zeros_fn,
        "sh": sh,
        "prev": None,
        "xbuf": np.empty((NCORES * P, FT), np.uint8),
        "qf": np.empty((NCORES * P, NBLK, BLK), np.float32),
        "ybuf": np.empty(NCORES * LOCAL, np.float32),
    }
    _CACHE["exec"] = state
    return state


def _quantize(x: np.ndarray, xbuf: np.ndarray, qf: np.ndarray) -> None:
    """Quantize x into the preallocated wire buffer xbuf [1024, FT] u8.

    numpy ties a fused numba kernel here (~37ms; memory-bandwidth-bound on
    this single-CPU host), so stay dependency-free.
    """
    xr = x.reshape(NCORES * P, NBLK, BLK)
    # absmax via max/-min: two reduction passes, no 64MB |x| temp
    absmax = np.maximum(xr.max(axis=2), -xr.min(axis=2))
    np.maximum(absmax, np.float32(1e-30), out=absmax)
    s = (absmax / np.float32(126.0)).astype(np.float32)
    inv = np.float32(1.0) / s
    np.multiply(xr, inv[..., None], out=qf)
    qf += np.float32(128.5)
    np.copyto(
        xbuf[:, :F].reshape(NCORES * P, NBLK, BLK), qf, casting="unsafe"
    )  # trunc of positive = floor = round-half-up
    xbuf[:, F:FT] = s.view(np.uint8).reshape(NCORES * P, SCOLS)


def _dequant_shard(Yj: np.ndarray, out: np.ndarray) -> None:
    """Dequantize one core's wire shard Yj [P, FT] u8 into out [LOCAL] f32."""
    o3 = out.reshape(P, NBLK, BLK)
    np.copyto(o3, Yj[:, :F].reshape(P, NBLK, BLK), casting="unsafe")
    o3 -= np.float32(128.0)
    o3 *= Yj[:, F:FT].copy().view(np.float32).reshape(P, NBLK, 1)


def run(x: np.ndarray, trace: bool = False):
    import jax

    st = _get_exec()
    x = np.ascontiguousarray(x, dtype=np.float32)
    assert x.shape == (NCORES * LOCAL,)

    _quantize(x, st["xbuf"], st["qf"])
    Xd = jax.device_put(st["xbuf"], st["sh"])

    bufs = st["zeros_fn"]() if st["prev"] is None else st["prev"]
    outs = st["sharded"](Xd, *bufs)
    st["prev"] = outs

    y_g = outs[0]
    y_g.copy_to_host_async()
    yv = st["ybuf"]
    shards = sorted(y_g.addressable_shards, key=lambda sh_: sh_.index[0].start)
    for j, shd in enumerate(shards):
        Yj = np.asarray(shd.data)
        _dequant_shard(Yj, yv[j * LOCAL: (j + 1) * LOCAL])
    # yv is a reused buffer: safe because callers consume the result before
    # invoking run() again (and kernel() returns the final attempt's view).
    return yv, _Res()


def kernel(Hamiltonian: np.ndarray) -> np.ndarray:
    # Validate cheaply on the host (no NaN + Parseval for the orthogonal
    # scaled transform) and retry on rare infra flakes.
    x = np.ascontiguousarray(Hamiltonian, dtype=np.float32)
    ref_norm2 = float(np.square(x, dtype=np.float64).sum()) / (NCORES * LOCAL)
    y = None
    for _attempt in range(3):
        try:
            y, _ = run(x, trace=False)
        except Exception:
            _CACHE.pop("exec", None)
            if _attempt == 2 and y is None:
                raise
            continue
        if np.isnan(y).any():
            continue
        norm2 = float(np.square(y, dtype=np.float64).sum())
        if abs(norm2 - ref_norm2) <= 0.02 * ref_norm2:
            break
    # run() returns a reused buffer view; hand the caller an owned copy.
    return None if y is None else y.copy()
